# revision 4
# baseline (speedup 1.0000x reference)
"""Transformer decoder layer (causal self-attn + cross-attn + FFN, 3 post-LNs)
on 8 Trainium2 NeuronCores — v2.

Sharding: 2-way data parallel (batch) x 4-way tensor parallel, with
sequence-parallel LayerNorms (Megatron-SP style):
  core c: batch g = c // 4, TP rank r = c % 4, token slice = [512r, 512r+512).
  - SA/CA attention: 4 of 16 heads per core (wq/wk/wv col slice 256, wo row
    slice). Output-projection partials -> ReduceScatter -> per-core token
    slice -> LayerNorm on 512 tokens only.
  - After LN1 the slice is transposed on-chip (PE) and AllGathered in
    feature-major layout, so CA's q-projection needs no DMA transpose.
  - FFN is data-parallel over tokens: full w1/w2 are streamed from DRAM in
    128-row chunks, each core computes its 512-token slice end to end; no
    collective after the FFN. LN3 is local; output slice written directly.
  - Collectives: RS1 (4MB bf16), AG (1MB->4MB bf16), RS2 (4MB bf16).

Attention inner kernel: scores matmul -> bf16 PSUM pairs -> one Exp per 2
k-blocks; AV computed d-major (v as stationary [128,65] with a ones column)
giving feature-major o directly plus the softmax denominator in row 64;
normalization via reciprocal + PE partition-broadcast + fused multiply on the
PSUM eviction. No o-transpose step. SA out-proj bias is folded into the
residual host-side; CA/FFN biases are added in the LN combine steps.
"""

import numpy as np
import ml_dtypes

import concourse.bass as bass
import concourse.bacc as bacc
import concourse.tile as tile
from concourse import mybir
from concourse import bass_utils
from concourse.masks import make_identity

F32 = mybir.dt.float32
BF16 = mybir.dt.bfloat16
AF = mybir.ActivationFunctionType
ALU = mybir.AluOpType

E = 1024
H_PER_CORE = 4
DK = 64
QKV = H_PER_CORE * DK    # 256
FFH = 4096               # full FFN hidden
EB = E // 128            # 8
G = 4                    # TP group size


def _ts(i, n):
    return slice(i * n, (i + 1) * n)


def _pbcast(ap, p=128):
    return bass.AP(tensor=ap.tensor, offset=ap.offset, ap=[[0, p]] + list(ap.ap))


PHASES = ["xt", "saqkv", "saattn", "sa", "cakv", "rs1", "ln1", "ag",
          "qproj", "ca", "rs2", "ln2", "ffn1", "full"]


def build_decoder_nc(S: int, num_devices: int = 8, stop_after: str | None = None,
                     repeat: int = 1, debug_outputs: bool = False):
    assert S % 512 == 0
    SL = S // G              # local token slice
    nc = bacc.Bacc("TRN2", target_bir_lowering=False, debug=False,
                   num_devices=num_devices)

    din = {}

    def inp(name, shape, dt):
        din[name] = nc.dram_tensor(name, list(shape), dt, kind="ExternalInput")
        return din[name]

    inp("x0_b", [S, E], BF16)
    inp("enc_b", [S, E], BF16)
    inp("res1", [SL, E], F32)             # x0 slice + sa_bo
    for p in ("sa", "ca"):
        inp(f"{p}_wq", [E, QKV], BF16)
        inp(f"{p}_wk", [E, QKV], BF16)
        inp(f"{p}_wv", [E, QKV], BF16)
        inp(f"{p}_wo", [H_PER_CORE, DK, E], BF16)   # head-major rows
        inp(f"{p}_bq", [QKV], F32)
        inp(f"{p}_bk", [QKV], F32)
    inp("ca_bo", [E], BF16)
    inp("w1", [E, FFH], BF16)
    inp("b1", [FFH], F32)
    inp("w2", [FFH, E], BF16)
    inp("b2", [E], BF16)
    for i in (1, 2, 3):
        inp(f"ln{i}_g", [E], BF16)
        inp(f"ln{i}_b", [E], BF16)
    inp("cmask", [4, 128, 512], BF16)

    out = nc.dram_tensor("out", [SL, E], F32, kind="ExternalOutput")

    rg = [[0, 1, 2, 3], [4, 5, 6, 7]][: max(1, num_devices // 4)]
    if num_devices < 8:
        rg = [list(range(num_devices))]

    dbg = {}
    if debug_outputs:
        for nm, shape in (("d_ar1", [S, E]), ("d_rs1", [SL, E]),
                          ("d_ago", [G, 128, EB, SL]), ("d_ar2", [S, E]),
                          ("d_rs2", [SL, E]), ("d_qT", [128, 2, S]),
                          ("d_oT", [64, H_PER_CORE, S])):
            dbg[nm] = nc.dram_tensor(nm, shape, BF16, kind="ExternalOutput")

    with tile.TileContext(nc) as tc:
        for _ in range(repeat):
            _emit(tc, din, out, S, rg, stop_after, dbg)

    nc.compile()
    return nc


def _emit(tc, din, out, S, rg, stop_after=None, dbg={}):
    nc = tc.nc
    SL = S // G
    TB = S // 128
    QT = S // 512
    SLTB = SL // 128         # 4
    CH = S // 512            # input stream chunks

    def cut(phase):
        return stop_after == phase

    with (
        tc.tile_pool(name="const", bufs=1) as const,
        tc.tile_pool(name="wpool", bufs=1) as wpool,
        tc.tile_pool(name="wstream", bufs=2) as wstream,
        tc.tile_pool(name="xchunk", bufs=2) as xchunk,
        tc.tile_pool(name="qkv", bufs=1) as qkv_pool,
        tc.tile_pool(name="opool", bufs=1) as o_pool,
        tc.tile_pool(name="atpool", bufs=2) as at_pool,
        tc.tile_pool(name="hpool", bufs=1) as h_pool,
        tc.tile_pool(name="xres", bufs=1) as xres_pool,
        tc.tile_pool(name="lnp", bufs=2) as lnp,
        tc.tile_pool(name="stat", bufs=8) as stat,
        tc.tile_pool(name="rcpp", bufs=2) as rcpp,
        tc.tile_pool(name="dram", bufs=1, space="DRAM") as dram,
    ):
        # ---------------- constants ----------------
        ident = const.tile([128, 128], BF16)
        make_identity(nc, ident)
        eps_t = const.tile([128, 1], F32)
        nc.vector.memset(eps_t, 1e-12)
        cmask = const.tile([128, 4, 512], BF16)
        nc.sync.dma_start(out=cmask, in_=din["cmask"].ap().rearrange("i p q -> p i q"))

        _bcc = {}

        def bcast(name, dt=BF16):
            if name not in _bcc:
                t = const.tile([128, E], dt, name=f"bc_{name}")
                nc.sync.dma_start(out=t, in_=_pbcast(din[name].ap()))
                _bcc[name] = t
            return _bcc[name]

        def pp_bias(name, nj):
            t = const.tile([128, nj], F32, name=f"ppb_{name}")
            nc.sync.dma_start(out=t, in_=din[name].ap().rearrange("(j p) -> p j", p=128))
            return t

        bq = {p: pp_bias(f"{p}_bq", 2) for p in ("sa", "ca")}
        bk = {p: pp_bias(f"{p}_bk", 2) for p in ("sa", "ca")}
        b1_t = pp_bias("b1", FFH // 128)

        # ---------------- DRAM scratch ----------------
        ar1_in = dram.tile([S, E], BF16, name="ar1_in")
        rs1_out = dram.tile([SL, E], BF16, name="rs1_out")
        ag_in_c = [dram.tile([128, EB, 128], BF16, name=f"ag_in{c}")
                   for c in range(QT)]
        ag_out_c = [dram.tile([G, 128, EB, 128], BF16, name=f"ag_out{c}")
                    for c in range(QT)]
        ar2_in = dram.tile([S, E], BF16, name="ar2_in")
        rs2_out = dram.tile([SL, E], BF16, name="rs2_out")

        # ---------------- weight loads ----------------
        def load_w_qkv(pref):
            w = {}
            for nm in ("wq", "wk", "wv"):
                t = wpool.tile([128, EB, QKV], BF16, tag=nm, name=f"{pref}_{nm}_sb")
                nc.sync.dma_start(out=t, in_=din[f"{pref}_{nm}"].ap().rearrange(
                    "(eb p) m -> p eb m", p=128))
                w[nm] = t
            return w

        def load_w_o(pref):
            t = wpool.tile([64, H_PER_CORE, E], BF16, tag="wo", name=f"{pref}_wo_sb")
            nc.sync.dma_start(out=t, in_=din[f"{pref}_wo"].ap().rearrange(
                "h p n -> p h n"))
            return t

        with (
            tc.tile_pool(name="pp", bufs=2, space="PSUM") as pp,
            tc.tile_pool(name="ss", bufs=2, space="PSUM") as ss,
            tc.tile_pool(name="po", bufs=2, space="PSUM") as po_p,
        ):
            def finish():
                nc.sync.dma_start(out=out.ap(), in_=din["res1"].ap())

            # ---------------- helpers ----------------
            def stream_chunks(src_dram, consume, nchunks=CH, name="xc"):
                # src [S, E] token-major DRAM -> per-chunk feature-major
                # [128, EB, 512] via transposing DMA; consume(s, xc)
                for s in range(nchunks):
                    xc = xchunk.tile([128, EB, 512], BF16, tag="xc",
                                     name=f"{name}{s}")
                    for eb in range(EB):
                        nc.sync.dma_start_transpose(
                            xc[:, eb, :], src_dram[_ts(s, 512), _ts(eb, 128)])
                    consume(s, xc)

            def proj_qk_chunk(s, xc, w, b, dst):
                # dst[:, j, W*s:...] = w.T @ xc + b  (feature-major)
                W = xc.shape[-1]
                for j in range(2):
                    ps = pp.tile([128, 512], F32, tag="pp")
                    for eb in range(EB):
                        nc.tensor.matmul(ps[:, 0:W], w[:, eb, _ts(j, 128)],
                                         xc[:, eb, :],
                                         start=(eb == 0), stop=(eb == EB - 1))
                    nc.vector.tensor_scalar_add(dst[:, j, _ts(s, W)], ps[:, 0:W],
                                                b[:, j:j + 1])

            def proj_v_chunk(s, xc, w, v):
                # v[:, 4s+i, h, d] token-major + ones col (bias folded host-side)
                for i in range(4):
                    ps = pp.tile([128, 512], F32, tag="pp")
                    for eb in range(EB):
                        nc.tensor.matmul(ps[:, 0:QKV], xc[:, eb, _ts(i, 128)],
                                         w[:, eb, :],
                                         start=(eb == 0), stop=(eb == EB - 1))
                    dst = v[:, 4 * s + i, :, 0:64]
                    src = ps[:, 0:QKV].rearrange("p (h d) -> p h d", d=64)
                    if i % 2 == 0:
                        nc.vector.tensor_copy(dst, src)
                    else:
                        nc.scalar.copy(dst, src)

            def attention(qT, kT, v, oT, causal, qts=None, on_qt_done=None,
                          local_qo=False):
                # oT [64, 4, S] feature-major per head; denominators via
                # ones column of v (row 64 of the AV psum). local_qo: qT/oT
                # are per-qt tiles with columns [0:512).
                for qt in (qts if qts is not None else range(QT)):
                    kb_max = min(TB, 4 * qt + 4) if causal else TB
                    qcol = slice(0, 512) if local_qo else _ts(qt, 512)
                    for h in range(H_PER_CORE):
                        hp = slice((h % 2) * 64, (h % 2) * 64 + 64)
                        j = h // 2
                        pov = po_p.tile([128, 512], F32, tag="po")
                        kb = 0
                        while kb < kb_max:
                            npass = min(4, kb_max - kb)
                            at = at_pool.tile([128, 4, 512], BF16, tag="at")
                            for i in range(0, npass, 2):
                                st = ss.tile([128, 2, 512], F32, tag="ss")
                                for u in range(2):
                                    nc.tensor.matmul(
                                        st[:, u, :],
                                        kT[hp, j, _ts(kb + i + u, 128)],
                                        qT[hp, j, qcol],
                                        start=True, stop=True)
                                nc.scalar.activation(at[:, i:i + 2, :], st,
                                                     AF.Exp, scale=0.125)
                            if causal and kb <= 4 * qt < kb + npass:
                                lo = 4 * qt - kb
                                nc.vector.tensor_mul(at[:, lo:lo + 2, :],
                                                     at[:, lo:lo + 2, :],
                                                     cmask[:, 0:2, :])
                                nc.vector.tensor_mul(at[:, lo + 2:lo + 4, :],
                                                     at[:, lo + 2:lo + 4, :],
                                                     cmask[:, 2:4, :])
                            for i in range(npass):
                                nc.tensor.matmul(pov[0:65, :], v[:, kb + i, h, :],
                                                 at[:, i, :],
                                                 start=(kb + i == 0),
                                                 stop=(kb + i == kb_max - 1))
                            kb += npass
                        # denom (psum partition 64) -> SBUF, DMA to partition
                        # 0, reciprocal, broadcast (HW pbcast needs base-0 src)
                        den = rcpp.tile([65, 512], BF16, tag="den")
                        nc.vector.tensor_copy(den[64:65, :], pov[64:65, :])
                        rcp = rcpp.tile([64, 512], BF16, tag="rcp")
                        nc.sync.dma_start(out=rcp[0:1, :], in_=den[64:65, :])
                        with nc.allow_low_precision(reason="softmax denom rcp"):
                            nc.vector.reciprocal(rcp[0:1, :], rcp[0:1, :])
                        nc.gpsimd.partition_broadcast(rcp[0:64, :], rcp[0:1, :])
                        nc.vector.tensor_tensor(oT[0:64, h, qcol],
                                                pov[0:64, :], rcp[0:64, :],
                                                ALU.mult)
                    if on_qt_done is not None:
                        on_qt_done(qt)

            def out_proj_qt(oT, wo, ar_dst, qt, perm=None, local_qo=False):
                # partial y (no bias) for qt's 4 token blocks -> ar_dst rows
                for i in range(4):
                    tb = 4 * qt + i
                    col = i if local_qo else tb
                    dst_tb = tb if perm is None else perm(qt, i)
                    y = lnp.tile([128, E], BF16, tag="yev")
                    for nh in range(2):
                        ps = pp.tile([128, 512], F32, tag="pp")
                        for h in range(H_PER_CORE):
                            nc.tensor.matmul(ps, oT[0:64, h, _ts(col, 128)],
                                             wo[0:64, h, _ts(nh, 512)],
                                             start=(h == 0), stop=(h == 3))
                        if nh == 0:
                            nc.vector.tensor_copy(y[:, _ts(nh, 512)], ps)
                        else:
                            nc.scalar.copy(y[:, _ts(nh, 512)], ps)
                    nc.sync.dma_start(out=ar_dst[_ts(dst_tb, 128), :], in_=y)

            def ln_core(ld, gname, bname):
                st = stat.tile([128, 2, 6], F32, tag="bnst")
                for sg in range(2):
                    nc.vector.bn_stats(st[:, sg, :], ld[:, _ts(sg, 512)])
                mv = stat.tile([128, 2], F32, tag="bnmv")
                nc.vector.bn_aggr(mv, st)
                sd = stat.tile([128, 1], F32, tag="sd")
                nc.scalar.activation(sd, mv[:, 1:2], AF.Sqrt, bias=eps_t)
                rstd = stat.tile([128, 1], F32, tag="rstd")
                nc.vector.reciprocal(rstd, sd)
                nc.vector.tensor_scalar(ld, ld, mv[:, 0:1], rstd,
                                        ALU.subtract, ALU.mult)
                nc.vector.tensor_mul(ld, ld, bcast(gname))
                nc.vector.tensor_add(ld, ld, bcast(bname))

            def layer_norm_slice(gname, bname, rs_src, combine, xT_dst,
                                 post=None):
                # combine(tb, arb) -> ld f32 [128, E] (residual + rs partial)
                # post(tb, ld): optional extra consumer of the LN output
                # xT_dst [128, EB, SL]: PE-transposed bf16 LN output
                for tb in range(SLTB):
                    arb = lnp.tile([128, E], BF16, tag="ln_bf")
                    nc.sync.dma_start(out=arb, in_=rs_src[_ts(tb, 128), :])
                    ld = combine(tb, arb)
                    ln_core(ld, gname, bname)
                    if post is not None:
                        post(tb, ld)
                    xb = lnp.tile([128, E], BF16, tag="ln_xb")
                    nc.scalar.copy(xb, ld)
                    for eb in range(EB):
                        pt = pp.tile([128, 128], BF16, tag="pp")
                        nc.tensor.transpose(pt, xb[:, _ts(eb, 128)], ident)
                        if eb % 2 == 0:
                            nc.vector.tensor_copy(xT_dst[:, eb, _ts(tb, 128)], pt)
                        else:
                            nc.scalar.copy(xT_dst[:, eb, _ts(tb, 128)], pt)

            # ================= SA =================
            sa_w = load_w_qkv("sa")
            sa_wo = load_w_o("sa")

            qT = qkv_pool.tile([128, 2, S], BF16, tag="qT", name="sa_qT")
            kT = qkv_pool.tile([128, 2, S], BF16, tag="kT", name="sa_kT")
            v = qkv_pool.tile([128, TB, 4, 65], BF16, tag="v", name="sa_v")
            nc.vector.memset(v[:, :, :, 64:65], 1.0)

            def sa_consume(s, xc):
                proj_qk_chunk(s, xc, sa_w["wq"], bq["sa"], qT)
                proj_qk_chunk(s, xc, sa_w["wk"], bk["sa"], kT)
                proj_v_chunk(s, xc, sa_w["wv"], v)

            stream_chunks(din["x0_b"].ap(), sa_consume, name="x0c")
            if cut("saqkv") or cut("xt"):
                finish()
                return

            oT = o_pool.tile([64, H_PER_CORE, S], BF16, tag="oT", name="sa_oT")
            attention(qT, kT, v, oT, causal=True,
                      on_qt_done=lambda qt: out_proj_qt(oT, sa_wo, ar1_in, qt))
            if cut("saattn") or cut("sa"):
                finish()
                return
            if dbg:
                nc.sync.dma_start(out=dbg["d_ar1"].ap(), in_=ar1_in[:, :])
                nc.sync.dma_start(out=dbg["d_qT"].ap(), in_=qT)
                nc.sync.dma_start(out=dbg["d_oT"].ap(), in_=oT)

            # ---- CA k/v from encoder (overlaps RS1/LN1/AG window) ----
            ca_w = load_w_qkv("ca")
            ca_wo = load_w_o("ca")
            ca_kT = qkv_pool.tile([128, 2, S], BF16, tag="kT", name="ca_kT")
            ca_v = qkv_pool.tile([128, TB, 4, 65], BF16, tag="v", name="ca_v")
            nc.vector.memset(ca_v[:, :, :, 64:65], 1.0)

            def ca_consume(s, xc):
                proj_qk_chunk(s, xc, ca_w["wk"], bk["ca"], ca_kT)
                proj_v_chunk(s, xc, ca_w["wv"], ca_v)

            stream_chunks(din["enc_b"].ap(), ca_consume, name="encc")
            if cut("cakv"):
                finish()
                return

            nc.gpsimd.collective_compute(
                "ReduceScatter", ALU.add, replica_groups=rg,
                ins=[ar1_in.opt()], outs=[rs1_out.opt()])
            if dbg:
                nc.sync.dma_start(out=dbg["d_rs1"].ap(), in_=rs1_out[:, :])
            if cut("rs1"):
                finish()
                return

            # ---- LN1 (sharded) + transpose + AG ----
            x1res = xres_pool.tile([128, SLTB, E], F32, tag="x1res")
            x1ts = xchunk.tile([128, EB, SL], BF16, tag="xc", name="x1ts")

            def ln1_combine(tb, arb):
                ld = x1res[:, tb, :]
                nc.sync.dma_start(out=ld, in_=din["res1"].ap()[_ts(tb, 128), :])
                nc.vector.tensor_add(ld, ld, arb)
                return ld

            layer_norm_slice("ln1_g", "ln1_b", rs1_out, ln1_combine, x1ts)
            for c in range(QT):
                nc.sync.dma_start(out=ag_in_c[c][:, :, :],
                                  in_=x1ts[:, :, _ts(c, 128)])
            if cut("ln1"):
                finish()
                return

            # chunk c gathers token block c of every slice: qt tile c holds
            # global token-blocks (QT*i + c) for src i; out_proj permutes back
            for c in range(QT):
                nc.gpsimd.collective_compute(
                    "AllGather", ALU.bypass, replica_groups=rg,
                    ins=[ag_in_c[c].opt()], outs=[ag_out_c[c].opt()])
            if dbg:
                for c in range(QT):
                    nc.sync.dma_start(
                        out=dbg["d_ago"].ap()[:, :, :, _ts(c, 128)],
                        in_=ag_out_c[c][:, :, :, :])
            if cut("ag"):
                finish()
                return

            ca_perm = lambda qt, i: QT * i + qt

            qcs = []
            for c in range(QT):
                qc = xchunk.tile([128, EB, 512], BF16, tag="xc", name=f"qc{c}")
                for si in range(G):
                    nc.sync.dma_start(out=qc[:, :, _ts(si, 128)],
                                      in_=ag_out_c[c][si, :, :, :])
                qcs.append(qc)
            for c in range(QT):
                qT_c = qkv_pool.tile([128, 2, 512], BF16, tag="qTc", bufs=2,
                                     name=f"ca_qT{c}")
                oT_c = o_pool.tile([64, H_PER_CORE, 512], BF16, tag="oTc",
                                   bufs=1, name=f"ca_oT{c}")
                proj_qk_chunk(0, qcs[c], ca_w["wq"], bq["ca"], qT_c)
                attention(qT_c, ca_kT, ca_v, oT_c, causal=False, qts=[c],
                          local_qo=True,
                          on_qt_done=lambda qt: out_proj_qt(
                              oT_c, ca_wo, ar2_in, qt, perm=ca_perm,
                              local_qo=True))
            if cut("qproj") or cut("ca"):
                finish()
                return
            if dbg:
                nc.sync.dma_start(out=dbg["d_ar2"].ap(), in_=ar2_in[:, :])

            nc.gpsimd.collective_compute(
                "ReduceScatter", ALU.add, replica_groups=rg,
                ins=[ar2_in.opt()], outs=[rs2_out.opt()])
            if dbg:
                nc.sync.dma_start(out=dbg["d_rs2"].ap(), in_=rs2_out[:, :])
            if cut("rs2"):
                finish()
                return

            # ---- LN2 (sharded) + transpose ----
            cabo_b = bcast("ca_bo")
            b2_b = bcast("b2")
            x2b = xres_pool.tile([128, SLTB, E], BF16, tag="x2b")
            x2ts = xchunk.tile([128, EB, SL], BF16, tag="xc", name="x2ts")

            def ln2_combine(tb, arb):
                ld = lnp.tile([128, E], F32, tag="ln_io")
                nc.vector.tensor_add(ld, x1res[:, tb, :], arb)
                nc.vector.tensor_add(ld, ld, cabo_b)
                return ld

            def ln2_post(tb, ld):
                nc.vector.tensor_add(x2b[:, tb, :], ld, b2_b)

            layer_norm_slice("ln2_g", "ln2_b", rs2_out, ln2_combine, x2ts,
                             post=ln2_post)
            if cut("ln2"):
                finish()
                return

            # ---- FFN1: hT = relu(w1.T x2T + b1), w1 streamed ----
            hT = h_pool.tile([128, FFH // 128, SL], BF16, tag="hT")
            for hb in range(FFH // 128):
                w1c = wstream.tile([128, EB, 128], BF16, tag="w1c")
                nc.sync.dma_start(out=w1c, in_=din["w1"].ap()[:, _ts(hb, 128)]
                                  .rearrange("(eb p) m -> p eb m", p=128))
                ps = pp.tile([128, 512], F32, tag="pp")
                for eb in range(EB):
                    nc.tensor.matmul(ps[:, 0:SL], w1c[:, eb, :], x2ts[:, eb, :],
                                     start=(eb == 0), stop=(eb == EB - 1))
                nc.scalar.activation(hT[:, hb, :], ps[:, 0:SL], AF.Relu,
                                     bias=b1_t[:, hb:hb + 1])
            if cut("ffn1"):
                finish()
                return

        # ---- FFN2 (8-bank PSUM accumulator, w2 streamed) + LN3 + out ----
        with tc.tile_pool(name="f2", bufs=1, space="PSUM") as f2_p:
            f2 = f2_p.tile([128, 2 * SLTB, 512], F32, tag="f2")
            for hb in range(FFH // 128):
                w2c = wstream.tile([128, E], BF16, tag="w2c")
                nc.sync.dma_start(out=w2c, in_=din["w2"].ap()[_ts(hb, 128), :])
                for tb in range(SLTB):
                    for nh in range(2):
                        nc.tensor.matmul(f2[:, 2 * tb + nh, :],
                                         hT[:, hb, _ts(tb, 128)],
                                         w2c[:, _ts(nh, 512)],
                                         start=(hb == 0), stop=(hb == FFH // 128 - 1))
            g3, b3 = bcast("ln3_g"), bcast("ln3_b")
            for tb in range(SLTB):
                ld = lnp.tile([128, E], F32, tag="ln_io")
                nc.vector.tensor_add(
                    ld, f2[:, 2 * tb:2 * tb + 2, :].rearrange("p a b -> p (a b)"),
                    x2b[:, tb, :])
                st = stat.tile([128, 2, 6], F32, tag="bnst")
                for sg in range(2):
                    nc.vector.bn_stats(st[:, sg, :], ld[:, _ts(sg, 512)])
                mv = stat.tile([128, 2], F32, tag="bnmv")
                nc.vector.bn_aggr(mv, st)
                sd = stat.tile([128, 1], F32, tag="sd")
                nc.scalar.activation(sd, mv[:, 1:2], AF.Sqrt, bias=eps_t)
                rstd = stat.tile([128, 1], F32, tag="rstd")
                nc.vector.reciprocal(rstd, sd)
                nc.vector.tensor_scalar(ld, ld, mv[:, 0:1], rstd,
                                        ALU.subtract, ALU.mult)
                nc.vector.tensor_mul(ld, ld, g3)
                nc.vector.tensor_add(ld, ld, b3)
                nc.sync.dma_start(out=out.ap()[_ts(tb, 128), :], in_=ld)


# ====================== host side ======================

def make_causal_masks():
    m = np.zeros((4, 128, 512), dtype=np.float32)
    pk = np.arange(128)[:, None]
    pq = np.arange(512)[None, :]
    for i in range(4):
        m[i] = (pk <= pq - 128 * i).astype(np.float32)
    return m.astype(ml_dtypes.bfloat16)


def shard_inputs(inputs, num_devices=8):
    bf = ml_dtypes.bfloat16
    f32 = np.float32
    cmask = make_causal_masks()
    inp = {k: np.asarray(v) for k, v in inputs.items()}
    S = inp["input"].shape[1]
    SL = S // G
    in_maps = []
    for c in range(num_devices):
        g = c // G if num_devices >= 8 else 0
        r = c % G
        qs = slice(r * QKV, (r + 1) * QKV)
        sl = slice(r * SL, (r + 1) * SL)
        x0 = inp["input"][g].astype(f32)
        # v-projection biases act uniformly on attention outputs (softmax
        # weights sum to 1), so bv @ wo folds into the post-attention bias.
        sa_bias = inp["sa_bo"] + inp["sa_bv"] @ inp["sa_wo"]
        ca_bias = inp["ca_bo"] + inp["ca_bv"] @ inp["ca_wo"]
        m = {
            "x0_b": x0.astype(bf),
            "enc_b": inp["encoder_output"][g].astype(bf),
            "res1": (x0[sl] + sa_bias[None, :]).astype(f32),
            "ca_bo": ca_bias.astype(bf),
            "w1": inp["ffn_w1"].astype(bf),
            "b1": inp["ffn_b1"].astype(f32),
            "w2": inp["ffn_w2"].astype(bf),
            "b2": inp["ffn_b2"].astype(bf),
            "cmask": cmask,
        }
        for p in ("sa", "ca"):
            m[f"{p}_wq"] = inp[f"{p}_wq"][:, qs].astype(bf)
            m[f"{p}_wk"] = inp[f"{p}_wk"][:, qs].astype(bf)
            m[f"{p}_wv"] = inp[f"{p}_wv"][:, qs].astype(bf)
            m[f"{p}_wo"] = inp[f"{p}_wo"][qs, :].reshape(4, 64, E).astype(bf)
            m[f"{p}_bq"] = inp[f"{p}_bq"][qs].astype(f32)
            m[f"{p}_bk"] = inp[f"{p}_bk"][qs].astype(f32)
        for i in (1, 2, 3):
            m[f"ln{i}_g"] = inp[f"ln{i}_g"].astype(bf)
            m[f"ln{i}_b"] = inp[f"ln{i}_b"].astype(bf)
        in_maps.append(m)
    return in_maps


_NC_CACHE = {}


def _get_nc(S):
    if S not in _NC_CACHE:
        _NC_CACHE[S] = build_decoder_nc(S)
    return _NC_CACHE[S]


def kernel(**inputs):
    x = np.asarray(inputs["input"])
    B, S, _ = x.shape
    nc = _get_nc(S)
    in_maps = shard_inputs(inputs)
    res = bass_utils.run_bass_kernel_spmd(nc, in_maps, core_ids=list(range(8)))
    outb = [np.concatenate([res.results[g * 4 + r]["out"] for r in range(4)], axis=0)
            for g in range(B)]
    return np.stack(outb, axis=0).astype(np.float32)
